# revision 12
# baseline (speedup 1.0000x reference)
"""GroupedAttention Trainium2 kernel.

Problem: x[2048, 2, 256]; K/V projections to G=2 groups (head width 256),
Q projection to G*SUB=8 heads; LayerNorm on K and Q; causal softmax
attention per (b, g, sub); output projection back to 256.

Sharding: 16 (b, g, sub) heads over 8 cores -> 2 heads per core.
Core c: b = c//4, g = (c//2)%2, sub-pair j = c%2 (subs 2j, 2j+1).
Each core computes its K/V projections (one (b,g) group), its two Q
heads, attention, and a partial output projection. The host sums the 4
partials per batch (the WO contraction is a sum over head slices) and
adds a folded constant bias (WO_b + sum_h V_bias_h @ WO_slice_h).

On-device layout: scores are computed transposed, ST[k, q] =
(KT chunk).T @ QT, so the post-softmax P[k, q] feeds the PV matmul
directly as the stationary operand (no transpose in the inner loop).
A ones-column appended to V makes PSUM column 256 accumulate the
softmax denominator for free; normalization folds into a per-partition
multiply after the output projection. Causal masking multiplies P by
one of four precomputed 0/1 masks (on GpSimd, which is otherwise idle).
LayerNorm mean arrives free via a host-appended -mean column in each
weight matrix; variance comes from one Square-activation with accum_out.
All matmuls run in float32r (1 cycle/row at moving>=256, vs 4 for fp32).
"""

import sys

import numpy as np

for _p in ("/opt/trn_rl_repo",):
    if _p not in sys.path:
        sys.path.insert(0, _p)

SEQ, BS, DIM = 2048, 2, 256
G, SUB = 2, 4
N_CORES = 8
LN_EPS = 1e-5
NT = SEQ // 128  # 16 seq tiles of 128
NSB = SEQ // 512  # 4 query superblocks of 512

_CACHE = {}


def _build_program():
    from contextlib import ExitStack

    import concourse.bacc as bacc
    import concourse.mybir as mybir
    from concourse import tile
    f32 = mybir.dt.float32
    f32r = mybir.dt.float32r
    AF = mybir.ActivationFunctionType
    OP = mybir.AluOpType

    nc = bacc.Bacc("TRN2", target_bir_lowering=False, debug=False)

    xt_d = nc.dram_tensor("xt", [128, 2, SEQ], f32r, kind="ExternalInput").ap()
    wk_d = nc.dram_tensor("wk", [128, 2, 258], f32r, kind="ExternalInput").ap()
    bk_d = nc.dram_tensor("bk", [1, 258], f32r, kind="ExternalInput").ap()
    wv_d = nc.dram_tensor("wv", [128, 2, 256], f32r, kind="ExternalInput").ap()
    wq_d = nc.dram_tensor("wq", [128, 4, 258], f32r, kind="ExternalInput").ap()
    bq_d = nc.dram_tensor("bq", [1, 2, 258], f32r, kind="ExternalInput").ap()
    wo_d = nc.dram_tensor("wo", [128, 4, 256], f32r, kind="ExternalInput").ap()
    lng_d = nc.dram_tensor("lng", [128, 2], f32, kind="ExternalInput").ap()
    id_d = nc.dram_tensor("ident", [128, 128], f32r, kind="ExternalInput").ap()
    cm_d = nc.dram_tensor("cmask", [128, 4, 512], f32r, kind="ExternalInput").ap()
    on_d = nc.dram_tensor("onesrow", [1, SEQ], f32r, kind="ExternalInput").ap()
    vo_d = nc.dram_tensor("vpones", [128, NT, 2], f32r, kind="ExternalInput").ap()
    out_d = nc.dram_tensor("out_partial", [SEQ, DIM], f32, kind="ExternalOutput").ap()

    r = lambda ap: ap.bitcast(f32r)

    with tile.TileContext(nc) as tc, ExitStack() as ctx:
        const = ctx.enter_context(tc.tile_pool(name="const", bufs=1))

        xt_sb = const.tile([128, 2, SEQ], f32r)
        wk_sb = const.tile([128, 2, 258], f32r)
        bk_sb = const.tile([1, 258], f32r)
        wv_sb = const.tile([128, 2, 256], f32r)
        wq_sb = const.tile([128, 4, 258], f32r)
        bq_sb = const.tile([1, 2, 258], f32r)
        wo_sb = const.tile([128, 4, 256], f32r)
        lng_sb = const.tile([128, 2], f32)
        kt_sb = const.tile([128, 2, SEQ], f32r)
        qt_sb = const.tile([128, 4, SEQ], f32r)
        vp_sb = const.tile([128, NT, 258], f32r)
        ot_sb = const.tile([128, 4, SEQ], f32r)
        masks_sb = const.tile([128, 4, 512], f32r)
        ident_sb = const.tile([128, 128], f32r)
        ones_sb = const.tile([1, SEQ], f32r)
        zero_sb = const.tile([128, 1], f32)
        eps_sb = const.tile([128, 1], f32)

        nc.sync.dma_start(xt_sb[:], xt_d[:])
        nc.sync.dma_start(wk_sb[:], wk_d[:])
        nc.sync.dma_start(bk_sb[:], bk_d[:])
        nc.sync.dma_start(wv_sb[:], wv_d[:])
        nc.sync.dma_start(wq_sb[:], wq_d[:])
        nc.sync.dma_start(bq_sb[:], bq_d[:])
        nc.sync.dma_start(wo_sb[:], wo_d[:])
        nc.sync.dma_start(lng_sb[:], lng_d[:])
        nc.sync.dma_start(ident_sb[:], id_d[:])
        nc.sync.dma_start(masks_sb[:], cm_d[:])
        nc.sync.dma_start(ones_sb[:], on_d[:])
        nc.sync.dma_start(vp_sb[:, :, 256:258], vo_d[:])
        nc.gpsimd.memset(zero_sb[:], 0.0)
        nc.gpsimd.memset(eps_sb[:], LN_EPS)

        psA = ctx.enter_context(tc.tile_pool(name="psA", bufs=2, space="PSUM"))
        psB = ctx.enter_context(tc.tile_pool(name="psB", bufs=1, space="PSUM"))
        psT = ctx.enter_context(tc.tile_pool(name="psT", bufs=2, space="PSUM"))
        wrk = ctx.enter_context(tc.tile_pool(name="wrk", bufs=3))
        ppool = ctx.enter_context(tc.tile_pool(name="ppool", bufs=3))
        opool = ctx.enter_context(tc.tile_pool(name="opool", bufs=2))

        # ---- V projection (no bias on device, no LN): vp_sb[k, f] ----
        for t in range(NT):
            vps = psA.tile([128, 256], f32, tag="mm512", name=f"vps{t}")
            for c in range(2):
                nc.tensor.matmul(
                    vps[:],
                    lhsT=r(xt_sb[:, c, t * 128 : (t + 1) * 128]),
                    rhs=r(wv_sb[:, c, :]),
                    start=(c == 0),
                    stop=(c == 1),
                )
            nc.vector.tensor_copy(vp_sb[:, t, 0:256], vps[:])

        # ---- K / Qa / Qb projections + LayerNorm + transpose ----
        # (dest tile, dest chunk base, weight chunks, bias row)
        ln_heads = [
            (kt_sb, 0, [wk_sb[:, 0, :], wk_sb[:, 1, :]], bk_sb[0:1, :]),
            (qt_sb, 0, [wq_sb[:, 0, :], wq_sb[:, 1, :]], bq_sb[0:1, 0, :]),
            (qt_sb, 2, [wq_sb[:, 2, :], wq_sb[:, 3, :]], bq_sb[0:1, 1, :]),
        ]
        for hi, (dest, cbase, wchunks, brow) in enumerate(ln_heads):
            for t in range(NT):
                pps = psA.tile([128, 258], f32, tag="mm512", name=f"pps{hi}_{t}")
                nc.tensor.matmul(
                    pps[:],
                    lhsT=r(xt_sb[:, 0, t * 128 : (t + 1) * 128]),
                    rhs=r(wchunks[0]),
                    start=True,
                    stop=False,
                )
                nc.tensor.matmul(
                    pps[:],
                    lhsT=r(xt_sb[:, 1, t * 128 : (t + 1) * 128]),
                    rhs=r(wchunks[1]),
                    start=False,
                    stop=False,
                )
                nc.tensor.matmul(
                    pps[:],
                    lhsT=r(ones_sb[0:1, t * 128 : (t + 1) * 128]),
                    rhs=r(brow),
                    start=False,
                    stop=True,
                )
                # col 256 of pps = -mean (host appended -mean weight column)
                mneg = wrk.tile([128, 1], f32, tag="mneg", name=f"mneg{hi}_{t}")
                nc.vector.tensor_copy(mneg[:], pps[:, 256:257])
                sq = wrk.tile([128, 256], f32, tag="sq", name=f"sq{hi}_{t}")
                var = wrk.tile([128, 1], f32, tag="var", name=f"var{hi}_{t}")
                nc.scalar.activation(
                    sq[:], pps[:, 0:256], AF.Square, bias=mneg[:], accum_out=var[:]
                )
                std = wrk.tile([128, 1], f32, tag="std", name=f"std{hi}_{t}")
                nc.scalar.activation(
                    std[:], var[:], AF.Sqrt, bias=eps_sb[:], scale=1.0 / 256.0
                )
                rstd = wrk.tile([128, 1], f32, tag="rstd", name=f"rstd{hi}_{t}")
                nc.vector.reciprocal(rstd[:], std[:])
                kn = wrk.tile([128, 256], f32r, tag="kn", name=f"kn{hi}_{t}")
                nc.vector.tensor_scalar(
                    kn[:],
                    pps[:, 0:256],
                    scalar1=mneg[:],
                    scalar2=rstd[:],
                    op0=OP.add,
                    op1=OP.mult,
                )
                for c in range(2):
                    tp = psT.tile([128, 128], f32r, tag="tp", name=f"tp{hi}_{t}_{c}")
                    nc.tensor.transpose(
                        tp[:], r(kn[:, c * 128 : (c + 1) * 128]), r(ident_sb[:])
                    )
                    # fold ln_g (per-feature, now per-partition) into the copy
                    nc.vector.tensor_scalar_mul(
                        dest[:, cbase + c, t * 128 : (t + 1) * 128],
                        tp[:].bitcast(f32),
                        lng_sb[:, c : c + 1],
                    )

        # ---- attention, 2 heads ----
        for h in range(2):
            for s in range(NSB):
                n_k = 4 * (s + 1)
                oacc = [
                    psB.tile([128, 258], f32, tag=f"oacc{j}", name=f"oacc{h}_{s}_{j}")
                    for j in range(4)
                ]
                for kt in range(n_k):
                    st = psA.tile([128, 512], f32, tag="mm512", name=f"st{h}_{s}_{kt}")
                    for c in range(2):
                        nc.tensor.matmul(
                            st[:],
                            lhsT=r(kt_sb[:, c, kt * 128 : (kt + 1) * 128]),
                            rhs=r(qt_sb[:, h * 2 + c, s * 512 : (s + 1) * 512]),
                            start=(c == 0),
                            stop=(c == 1),
                        )
                    p = ppool.tile([128, 512], f32r, tag="p", name=f"p{h}_{s}_{kt}")
                    nc.scalar.activation(
                        p[:], st[:], AF.Exp, bias=zero_sb[:], scale=1.0 / 16.0
                    )
                    if kt >= n_k - 4:
                        o = kt - (n_k - 4)
                        nc.vector.tensor_mul(p[:], p[:], masks_sb[:, o, :])
                    for j in range(4):
                        nc.tensor.matmul(
                            oacc[j][:],
                            lhsT=r(p[:, j * 128 : (j + 1) * 128]),
                            rhs=r(vp_sb[:, kt, :]),
                            start=(kt == 0),
                            stop=(kt == n_k - 1),
                        )
                for j in range(4):
                    rc = wrk.tile([128, 1], f32, tag="rc", name=f"rc{h}_{s}_{j}")
                    nc.vector.reciprocal(rc[:], oacc[j][:, 256:257])
                    osb = opool.tile([128, 256], f32r, tag="osb", name=f"osb{h}_{s}_{j}")
                    nc.vector.tensor_scalar_mul(osb[:], oacc[j][:, 0:256], rc[:])
                    for c in range(2):
                        otp = psT.tile(
                            [128, 128], f32r, tag="tp", name=f"otp{h}_{s}_{j}_{c}"
                        )
                        nc.tensor.transpose(
                            otp[:], r(osb[:, c * 128 : (c + 1) * 128]), r(ident_sb[:])
                        )
                        nc.vector.tensor_copy(
                            ot_sb[
                                :,
                                h * 2 + c,
                                s * 512 + j * 128 : s * 512 + (j + 1) * 128,
                            ],
                            otp[:].bitcast(f32),
                        )

        # ---- output projection: OUT[s_tile, d] = sum_c OT[c, s_tile].T @ WO[c] ----
        for t in range(NT):
            ops = psB.tile([128, 256], f32, tag=f"oacc{t % 4}", name=f"ops{t}")
            for c in range(4):
                nc.tensor.matmul(
                    ops[:],
                    lhsT=r(ot_sb[:, c, t * 128 : (t + 1) * 128]),
                    rhs=r(wo_sb[:, c, :]),
                    start=(c == 0),
                    stop=(c == 3),
                )
            outsb = opool.tile([128, 256], f32, tag="outsb", name=f"outsb{t}")
            nc.scalar.copy(outsb[:], ops[:])
            nc.sync.dma_start(out_d[t * 128 : (t + 1) * 128, :], outsb[:])

    nc.finalize()
    return nc


def _chunk2(a):
    """[256, F] -> [128, 2, F] (feature chunks on the free axis)."""
    f = a.shape[1]
    return np.ascontiguousarray(a.reshape(2, 128, f).transpose(1, 0, 2))


def _prep_core_inputs(c, x, WK_w, WK_b, WV_w, WV_b, WQ_w, WQ_b, WO_w, ln_g):
    b, g, j = c // 4, (c // 2) % 2, c % 2
    f32 = np.float32

    xT = np.ascontiguousarray(x[:, b, :].T.astype(f32))  # [256, 2048]
    xt = _chunk2(xT)

    def ln_weight(w, bias):  # w [256, 256], bias [256] -> w' [256,258], b' [1,258]
        wm = -w.mean(axis=1, keepdims=True)
        zc = np.zeros_like(wm)
        wp = np.concatenate([w, wm, zc], axis=1).astype(f32)
        bp = np.concatenate([bias, [-bias.mean()], [0.0]]).astype(f32)[None, :]
        return wp, bp

    wk_s = WK_w[:, g * 256 : (g + 1) * 256]
    wkp, bkp = ln_weight(wk_s, WK_b[g * 256 : (g + 1) * 256])

    wv_s = np.ascontiguousarray(WV_w[:, g * 256 : (g + 1) * 256].astype(f32))

    wq_chunks, bq_rows = [], []
    for sh in (2 * j, 2 * j + 1):
        col = (g * SUB + sh) * 256
        wqp, bqp = ln_weight(WQ_w[:, col : col + 256], WQ_b[col : col + 256])
        wq_chunks.append(_chunk2(wqp))
        bq_rows.append(bqp)
    wq = np.ascontiguousarray(
        np.concatenate(
            [wq_chunks[0], wq_chunks[1]], axis=1
        )  # [128, 2, 257] + [128, 2, 257] -> [128, 4, 257]
    )
    bq = np.ascontiguousarray(np.stack([bq_rows[0][0], bq_rows[1][0]])[None, :, :])

    row = (g * SUB + 2 * j) * 256
    wo_s = WO_w[row : row + 512, :].astype(f32)  # [512, 256]
    wo = np.ascontiguousarray(wo_s.reshape(4, 128, 256).transpose(1, 0, 2))

    lng = np.ascontiguousarray(ln_g.astype(f32).reshape(2, 128).T)

    f = np.float32
    pp, ff = np.meshgrid(np.arange(128), np.arange(512), indexing="ij")
    cmask = np.stack(
        [(o * 128 + pp <= ff).astype(f) for o in range(4)], axis=1
    )  # [128, 4, 512]
    return {
        "xt": xt,
        "wk": _chunk2(wkp),
        "bk": bkp,
        "wv": _chunk2(wv_s),
        "wq": wq,
        "bq": bq,
        "wo": wo,
        "lng": lng,
        "ident": np.eye(128, dtype=f),
        "cmask": np.ascontiguousarray(cmask),
        "onesrow": np.ones((1, SEQ), dtype=f),
        "vpones": np.concatenate(
            [np.ones((128, NT, 1), dtype=f), np.zeros((128, NT, 1), dtype=f)], axis=2
        ),
    }


def kernel(x, WK_w, WK_b, WV_w, WV_b, WQ_w, WQ_b, WO_w, WO_b, ln_g, ln_b, **kwargs):
    x = np.asarray(x)
    WK_w, WK_b = np.asarray(WK_w), np.asarray(WK_b)
    WV_w, WV_b = np.asarray(WV_w), np.asarray(WV_b)
    WQ_w, WQ_b = np.asarray(WQ_w), np.asarray(WQ_b)
    WO_w, WO_b = np.asarray(WO_w), np.asarray(WO_b)
    ln_g, ln_b = np.asarray(ln_g), np.asarray(ln_b)

    if not np.allclose(ln_b, 0.0):
        raise NotImplementedError("nonzero ln_b not supported by this kernel")

    if "nc" not in _CACHE:
        _CACHE["nc"] = _build_program()
    nc = _CACHE["nc"]

    in_maps = [
        _prep_core_inputs(c, x, WK_w, WK_b, WV_w, WV_b, WQ_w, WQ_b, WO_w, ln_g)
        for c in range(N_CORES)
    ]

    from concourse.bass_utils import run_bass_kernel_spmd

    res = run_bass_kernel_spmd(nc, in_maps, list(range(N_CORES)))
    results = res.results

    out = np.zeros((SEQ, BS, DIM), dtype=np.float32)
    for c in range(N_CORES):
        out[:, c // 4, :] += results[c]["out_partial"]

    # fold: WO_b plus the V-bias contribution of every head
    const_bias = WO_b.astype(np.float64).copy()
    for g in range(G):
        bv = WV_b[g * 256 : (g + 1) * 256].astype(np.float64)
        for sh in range(SUB):
            row = (g * SUB + sh) * 256
            const_bias += bv @ WO_w[row : row + 256, :].astype(np.float64)
    out += const_bias.astype(np.float32)[None, None, :]
    return out


# revision 13
# speedup vs baseline: 1.0803x; 1.0803x over previous
"""GroupedAttention Trainium2 kernel.

Problem: x[2048, 2, 256]; K/V projections to G=2 groups (head width 256),
Q projection to G*SUB=8 heads; LayerNorm on K and Q; causal softmax
attention per (b, g, sub); output projection back to 256.

Sharding: 16 (b, g, sub) heads over 8 cores -> 2 heads per core.
Core c: b = c//4, g = (c//2)%2, sub-pair j = c%2 (subs 2j, 2j+1).
Each core computes its K/V projections (one (b,g) group), its two Q
heads, attention, and a partial output projection. The host sums the 4
partials per batch (the WO contraction is a sum over head slices) and
adds a folded constant bias (WO_b + sum_h V_bias_h @ WO_slice_h).

On-device layout: scores are computed transposed, ST[k, q] =
(KT chunk).T @ QT, so the post-softmax P[k, q] feeds the PV matmul
directly as the stationary operand (no transpose in the inner loop).
A ones-column appended to V makes PSUM column 256 accumulate the
softmax denominator for free; normalization folds into a per-partition
multiply after the output projection. Causal masking multiplies P by
one of four precomputed 0/1 masks (on GpSimd, which is otherwise idle).
LayerNorm mean arrives free via a host-appended -mean column in each
weight matrix; variance comes from one Square-activation with accum_out.
All matmuls run in float32r (1 cycle/row at moving>=256, vs 4 for fp32).
"""

import sys

import numpy as np

for _p in ("/opt/trn_rl_repo",):
    if _p not in sys.path:
        sys.path.insert(0, _p)

SEQ, BS, DIM = 2048, 2, 256
G, SUB = 2, 4
N_CORES = 8
LN_EPS = 1e-5
NT = SEQ // 128  # 16 seq tiles of 128
NSB = SEQ // 512  # 4 query superblocks of 512

_CACHE = {}


def _build_program():
    from contextlib import ExitStack

    import concourse.bacc as bacc
    import concourse.mybir as mybir
    from concourse import tile
    f32 = mybir.dt.float32
    f32r = mybir.dt.float32r
    AF = mybir.ActivationFunctionType
    OP = mybir.AluOpType

    nc = bacc.Bacc("TRN2", target_bir_lowering=False, debug=False)

    xt_d = nc.dram_tensor("xt", [128, 2, SEQ], f32r, kind="ExternalInput").ap()
    wk_d = nc.dram_tensor("wk", [128, 2, 258], f32r, kind="ExternalInput").ap()
    bk_d = nc.dram_tensor("bk", [1, 258], f32r, kind="ExternalInput").ap()
    wv_d = nc.dram_tensor("wv", [128, 2, 256], f32r, kind="ExternalInput").ap()
    wq_d = nc.dram_tensor("wq", [128, 4, 258], f32r, kind="ExternalInput").ap()
    bq_d = nc.dram_tensor("bq", [1, 2, 258], f32r, kind="ExternalInput").ap()
    wo_d = nc.dram_tensor("wo", [128, 4, 256], f32r, kind="ExternalInput").ap()
    lng_d = nc.dram_tensor("lng", [128, 2], f32, kind="ExternalInput").ap()
    id_d = nc.dram_tensor("ident", [128, 128], f32r, kind="ExternalInput").ap()
    cm_d = nc.dram_tensor("cmask", [128, 4, 512], f32r, kind="ExternalInput").ap()
    on_d = nc.dram_tensor("onesrow", [1, SEQ], f32r, kind="ExternalInput").ap()
    vo_d = nc.dram_tensor("vpones", [128, NT, 2], f32r, kind="ExternalInput").ap()
    out_d = nc.dram_tensor("out_partial", [SEQ, DIM], f32, kind="ExternalOutput").ap()

    r = lambda ap: ap.bitcast(f32r)

    with tile.TileContext(nc) as tc, ExitStack() as ctx:
        const = ctx.enter_context(tc.tile_pool(name="const", bufs=1))

        xt_sb = const.tile([128, 2, SEQ], f32r)
        wk_sb = const.tile([128, 2, 258], f32r)
        bk_sb = const.tile([1, 258], f32r)
        wv_sb = const.tile([128, 2, 256], f32r)
        wq_sb = const.tile([128, 4, 258], f32r)
        bq_sb = const.tile([1, 2, 258], f32r)
        wo_sb = const.tile([128, 4, 256], f32r)
        lng_sb = const.tile([128, 2], f32)
        ident_sb = const.tile([128, 128], f32r)
        ones_sb = const.tile([1, SEQ], f32r)
        zero_sb = const.tile([128, 1], f32)
        eps_sb = const.tile([128, 1], f32)

        # per-tile tensors for exact dependency tracking
        kt_t = [
            [const.tile([128, 128], f32r, name=f"ktt{c}_{t}") for t in range(NT)]
            for c in range(2)
        ]
        qt_t = [
            [const.tile([128, 512], f32r, name=f"qtt{ci}_{s}") for s in range(NSB)]
            for ci in range(4)
        ]
        vp_t = [const.tile([128, 258], f32r, name=f"vpt{t}") for t in range(NT)]
        ot_t = [
            [
                [const.tile([128, 128], f32r, name=f"ott{c}_{s}_{j}") for j in range(4)]
                for s in range(NSB)
            ]
            for c in range(4)
        ]
        masks_t = [const.tile([128, 512], f32r, name=f"mask{o}") for o in range(4)]

        nc.sync.dma_start(xt_sb[:], xt_d[:])
        nc.sync.dma_start(wk_sb[:], wk_d[:])
        nc.sync.dma_start(bk_sb[:], bk_d[:])
        nc.sync.dma_start(wv_sb[:], wv_d[:])
        nc.sync.dma_start(wq_sb[:], wq_d[:])
        nc.sync.dma_start(bq_sb[:], bq_d[:])
        nc.sync.dma_start(wo_sb[:], wo_d[:])
        nc.sync.dma_start(lng_sb[:], lng_d[:])
        nc.sync.dma_start(ident_sb[:], id_d[:])
        for o in range(4):
            nc.sync.dma_start(masks_t[o][:], cm_d[:, o, :])
        nc.sync.dma_start(ones_sb[:], on_d[:])
        for t in range(NT):
            nc.sync.dma_start(vp_t[t][:, 256:258], vo_d[:, t, :])
        nc.gpsimd.memset(zero_sb[:], 0.0)
        nc.gpsimd.memset(eps_sb[:], LN_EPS)

        psA = ctx.enter_context(tc.tile_pool(name="psA", bufs=3, space="PSUM"))
        psB = ctx.enter_context(tc.tile_pool(name="psB", bufs=1, space="PSUM"))
        psT = ctx.enter_context(tc.tile_pool(name="psT", bufs=1, space="PSUM"))
        wrk = ctx.enter_context(tc.tile_pool(name="wrk", bufs=3))
        ppool = ctx.enter_context(tc.tile_pool(name="ppool", bufs=3))
        opool = ctx.enter_context(tc.tile_pool(name="opool", bufs=2))

        def v_proj(t):
            vps = psA.tile([128, 256], f32, tag="mm512", name=f"vps{t}")
            for c in range(2):
                nc.tensor.matmul(
                    vps[:],
                    lhsT=r(xt_sb[:, c, t * 128 : (t + 1) * 128]),
                    rhs=r(wv_sb[:, c, :]),
                    start=(c == 0),
                    stop=(c == 1),
                )
            nc.vector.tensor_copy(vp_t[t][:, 0:256], vps[:])

        def ln_proj(hi, t, wchunks, brow, dest_write):
            """project seq-tile t, layernorm, transpose; dest_write(c, tp_psum)"""
            pps = psA.tile([128, 258], f32, tag="mm512", name=f"pps{hi}_{t}")
            nc.tensor.matmul(
                pps[:],
                lhsT=r(xt_sb[:, 0, t * 128 : (t + 1) * 128]),
                rhs=r(wchunks[0]),
                start=True,
                stop=False,
            )
            nc.tensor.matmul(
                pps[:],
                lhsT=r(xt_sb[:, 1, t * 128 : (t + 1) * 128]),
                rhs=r(wchunks[1]),
                start=False,
                stop=False,
            )
            nc.tensor.matmul(
                pps[:],
                lhsT=r(ones_sb[0:1, t * 128 : (t + 1) * 128]),
                rhs=r(brow),
                start=False,
                stop=True,
            )
            # col 256 of pps = -mean (host appended -mean weight column)
            mneg = wrk.tile([128, 1], f32, tag="mneg", name=f"mneg{hi}_{t}")
            nc.vector.tensor_copy(mneg[:], pps[:, 256:257])
            sq = wrk.tile([128, 256], f32, tag="sq", name=f"sq{hi}_{t}")
            var = wrk.tile([128, 1], f32, tag="var", name=f"var{hi}_{t}")
            nc.scalar.activation(
                sq[:], pps[:, 0:256], AF.Square, bias=mneg[:], accum_out=var[:]
            )
            std = wrk.tile([128, 1], f32, tag="std", name=f"std{hi}_{t}")
            nc.scalar.activation(
                std[:], var[:], AF.Sqrt, bias=eps_sb[:], scale=1.0 / 256.0
            )
            rstd = wrk.tile([128, 1], f32, tag="rstd", name=f"rstd{hi}_{t}")
            nc.vector.reciprocal(rstd[:], std[:])
            kn = wrk.tile([128, 256], f32r, tag="kn", name=f"kn{hi}_{t}")
            nc.vector.tensor_scalar(
                kn[:],
                pps[:, 0:256],
                scalar1=mneg[:],
                scalar2=rstd[:],
                op0=OP.add,
                op1=OP.mult,
            )
            for c in range(2):
                tp = psT.tile([128, 128], f32r, tag="tp", name=f"tp{hi}_{t}_{c}")
                nc.tensor.transpose(
                    tp[:], r(kn[:, c * 128 : (c + 1) * 128]), r(ident_sb[:])
                )
                dest_write(c, tp)

        def k_write(t):
            def w(c, tp):
                nc.vector.tensor_scalar_mul(
                    kt_t[c][t][:], tp[:].bitcast(f32), lng_sb[:, c : c + 1]
                )

            return w

        def q_write(cbase, t):
            def w(c, tp):
                nc.vector.tensor_scalar_mul(
                    qt_t[cbase + c][t // 4][:, (t % 4) * 128 : (t % 4 + 1) * 128],
                    tp[:].bitcast(f32),
                    lng_sb[:, c : c + 1],
                )

            return w

        # round-robin projections so attention can start after 4 seq-tiles
        for t in range(NT):
            ln_proj(0, t, [wk_sb[:, 0, :], wk_sb[:, 1, :]], bk_sb[0:1, :], k_write(t))
            ln_proj(
                1, t, [wq_sb[:, 0, :], wq_sb[:, 1, :]], bq_sb[0:1, 0, :], q_write(0, t)
            )
            v_proj(t)
            ln_proj(
                2, t, [wq_sb[:, 2, :], wq_sb[:, 3, :]], bq_sb[0:1, 1, :], q_write(2, t)
            )

        # ---- attention: heads interleaved at superblock granularity ----
        def attn_superblock(h, s):
            n_k = 4 * (s + 1)
            oacc = [
                psB.tile([128, 258], f32, tag=f"oacc{j}", name=f"oacc{h}_{s}_{j}")
                for j in range(4)
            ]
            for kt in range(n_k):
                st = psA.tile([128, 512], f32, tag="mm512", name=f"st{h}_{s}_{kt}")
                for c in range(2):
                    nc.tensor.matmul(
                        st[:],
                        lhsT=r(kt_t[c][kt][:]),
                        rhs=r(qt_t[h * 2 + c][s][:]),
                        start=(c == 0),
                        stop=(c == 1),
                    )
                p = ppool.tile([128, 512], f32r, tag="p", name=f"p{h}_{s}_{kt}")
                nc.scalar.activation(
                    p[:], st[:], AF.Exp, bias=zero_sb[:], scale=1.0 / 16.0
                )
                if kt >= n_k - 4:
                    o = kt - (n_k - 4)
                    nc.vector.tensor_mul(p[:], p[:], masks_t[o][:])
                for j in range(4):
                    nc.tensor.matmul(
                        oacc[j][:],
                        lhsT=r(p[:, j * 128 : (j + 1) * 128]),
                        rhs=r(vp_t[kt][:]),
                        start=(kt == 0),
                        stop=(kt == n_k - 1),
                    )
            for j in range(4):
                rc = wrk.tile([128, 1], f32, tag="rc", name=f"rc{h}_{s}_{j}")
                nc.vector.reciprocal(rc[:], oacc[j][:, 256:257])
                osb = opool.tile([128, 256], f32r, tag="osb", name=f"osb{h}_{s}_{j}")
                nc.vector.tensor_scalar_mul(osb[:], oacc[j][:, 0:256], rc[:])
                for c in range(2):
                    otp = psT.tile(
                        [128, 128], f32r, tag="tp", name=f"otp{h}_{s}_{j}_{c}"
                    )
                    nc.tensor.transpose(
                        otp[:], r(osb[:, c * 128 : (c + 1) * 128]), r(ident_sb[:])
                    )
                    nc.vector.tensor_copy(ot_t[h * 2 + c][s][j][:], otp[:].bitcast(f32))

        def o_proj(t):
            s, j = t // 4, t % 4
            ops = psB.tile([128, 256], f32, tag=f"oacc{t % 4}", name=f"ops{t}")
            for c in range(4):
                nc.tensor.matmul(
                    ops[:],
                    lhsT=r(ot_t[c][s][j][:]),
                    rhs=r(wo_sb[:, c, :]),
                    start=(c == 0),
                    stop=(c == 3),
                )
            outsb = opool.tile([128, 256], f32, tag="outsb", name=f"outsb{t}")
            nc.scalar.copy(outsb[:], ops[:])
            nc.sync.dma_start(out_d[t * 128 : (t + 1) * 128, :], outsb[:])

        for s in range(NSB):
            for h in range(2):
                attn_superblock(h, s)
            for t in range(4 * s, 4 * s + 4):
                o_proj(t)

    nc.finalize()
    return nc


def _chunk2(a):
    """[256, F] -> [128, 2, F] (feature chunks on the free axis)."""
    f = a.shape[1]
    return np.ascontiguousarray(a.reshape(2, 128, f).transpose(1, 0, 2))


def _prep_core_inputs(c, x, WK_w, WK_b, WV_w, WV_b, WQ_w, WQ_b, WO_w, ln_g):
    b, g, j = c // 4, (c // 2) % 2, c % 2
    f32 = np.float32

    xT = np.ascontiguousarray(x[:, b, :].T.astype(f32))  # [256, 2048]
    xt = _chunk2(xT)

    def ln_weight(w, bias):  # w [256, 256], bias [256] -> w' [256,258], b' [1,258]
        wm = -w.mean(axis=1, keepdims=True)
        zc = np.zeros_like(wm)
        wp = np.concatenate([w, wm, zc], axis=1).astype(f32)
        bp = np.concatenate([bias, [-bias.mean()], [0.0]]).astype(f32)[None, :]
        return wp, bp

    wk_s = WK_w[:, g * 256 : (g + 1) * 256]
    wkp, bkp = ln_weight(wk_s, WK_b[g * 256 : (g + 1) * 256])

    wv_s = np.ascontiguousarray(WV_w[:, g * 256 : (g + 1) * 256].astype(f32))

    wq_chunks, bq_rows = [], []
    for sh in (2 * j, 2 * j + 1):
        col = (g * SUB + sh) * 256
        wqp, bqp = ln_weight(WQ_w[:, col : col + 256], WQ_b[col : col + 256])
        wq_chunks.append(_chunk2(wqp))
        bq_rows.append(bqp)
    wq = np.ascontiguousarray(
        np.concatenate(
            [wq_chunks[0], wq_chunks[1]], axis=1
        )  # [128, 2, 257] + [128, 2, 257] -> [128, 4, 257]
    )
    bq = np.ascontiguousarray(np.stack([bq_rows[0][0], bq_rows[1][0]])[None, :, :])

    row = (g * SUB + 2 * j) * 256
    wo_s = WO_w[row : row + 512, :].astype(f32)  # [512, 256]
    wo = np.ascontiguousarray(wo_s.reshape(4, 128, 256).transpose(1, 0, 2))

    lng = np.ascontiguousarray(ln_g.astype(f32).reshape(2, 128).T)

    f = np.float32
    pp, ff = np.meshgrid(np.arange(128), np.arange(512), indexing="ij")
    cmask = np.stack(
        [(o * 128 + pp <= ff).astype(f) for o in range(4)], axis=1
    )  # [128, 4, 512]
    return {
        "xt": xt,
        "wk": _chunk2(wkp),
        "bk": bkp,
        "wv": _chunk2(wv_s),
        "wq": wq,
        "bq": bq,
        "wo": wo,
        "lng": lng,
        "ident": np.eye(128, dtype=f),
        "cmask": np.ascontiguousarray(cmask),
        "onesrow": np.ones((1, SEQ), dtype=f),
        "vpones": np.concatenate(
            [np.ones((128, NT, 1), dtype=f), np.zeros((128, NT, 1), dtype=f)], axis=2
        ),
    }


def kernel(x, WK_w, WK_b, WV_w, WV_b, WQ_w, WQ_b, WO_w, WO_b, ln_g, ln_b, **kwargs):
    x = np.asarray(x)
    WK_w, WK_b = np.asarray(WK_w), np.asarray(WK_b)
    WV_w, WV_b = np.asarray(WV_w), np.asarray(WV_b)
    WQ_w, WQ_b = np.asarray(WQ_w), np.asarray(WQ_b)
    WO_w, WO_b = np.asarray(WO_w), np.asarray(WO_b)
    ln_g, ln_b = np.asarray(ln_g), np.asarray(ln_b)

    if not np.allclose(ln_b, 0.0):
        raise NotImplementedError("nonzero ln_b not supported by this kernel")

    if "nc" not in _CACHE:
        _CACHE["nc"] = _build_program()
    nc = _CACHE["nc"]

    in_maps = [
        _prep_core_inputs(c, x, WK_w, WK_b, WV_w, WV_b, WQ_w, WQ_b, WO_w, ln_g)
        for c in range(N_CORES)
    ]

    from concourse.bass_utils import run_bass_kernel_spmd

    res = run_bass_kernel_spmd(nc, in_maps, list(range(N_CORES)))
    results = res.results

    out = np.zeros((SEQ, BS, DIM), dtype=np.float32)
    for c in range(N_CORES):
        out[:, c // 4, :] += results[c]["out_partial"]

    # fold: WO_b plus the V-bias contribution of every head
    const_bias = WO_b.astype(np.float64).copy()
    for g in range(G):
        bv = WV_b[g * 256 : (g + 1) * 256].astype(np.float64)
        for sh in range(SUB):
            row = (g * SUB + sh) * 256
            const_bias += bv @ WO_w[row : row + 256, :].astype(np.float64)
    out += const_bias.astype(np.float32)[None, None, :]
    return out
